# revision 21
# baseline (speedup 1.0000x reference)
"""Trainium2 8-core kernel for nn_MultiHeadAttention_83408264889124.

Full inputs in, full output out. Sharding: batch (4) x head-group (2) grid
over 8 NeuronCores — each core computes one batch with 6 of the 12 heads and
produces a partial Y^T = sum_h W_o[h]^T @ O_h^T; the host adds the two
head-group partials per batch (the "all-reduce" of the TP split) and
transposes back. All device work is in transposed layouts so no on-device
transposes are needed:

  Qt = (wq/sqrt(K))^T X^T, Kt = wk^T X^T          [K, S] per head
  St[k,q] = sum_d Kt[d,k] Qt[d,q]                  (2-head row-packed matmuls)
  E = exp(St)  (ScalarE, PSUM->SBUF bf16)
  AV with stationary [V_h | ones*64]: U[0:64] = V^T E, U[64:128] = colsum(E)
  Ot = U[0:64] * recip(U[64:128])                  (VectorE)

The reference does a RAW reshape [B,H,S,V] -> [B,S,H*V] (no transpose) before
W_o, which scrambles (head, seq): with t = S*h + s, output row s' = t//H gets
feature block j = t%H from head h, position s. Because S*HPC/H = 1024 exactly,
each head-group core produces a clean half of the output rows, and the scatter
indices depend only on the LOCAL head index - so one SPMD program works for
all cores. The output projection is Y^T = W_o^T @ G^T with the FULL W_o.

Schedule (the perf-critical part): the ScalarE exp stream is the bottleneck
resource (192 ACTIVATEs x ~1.1us with zero slack). The kernel therefore
minimizes time outside the exp-saturated window:
  - DMA order: wk, xk, wv, xv, wq, xq[c0], xq[c1:], wo; all DRAM tensors are
    host-packed partition-major ([128, ...] with large contiguous runs) so
    DMA runs at full rate.
  - Prologue (overlapped with DMA): PE warm-up burst, K-proj pair0 chunk-by-
    chunk behind the xk chunk DMAs, V-proj all kt behind xv, Q-proj pair0
    chunk0. First exp issues ~27us in.
  - All remaining projections (K/Q pairs 1-2, Q pair0 chunks 1-3) and the
    first output-projection column chunk (W_o c0 depends only on heads 0-2 =
    pairs 0,1) are emitted as single-matmul micro-units interleaved ~2 per kt
    iteration into the attention loops, filling the PE slack under the exp
    stream. Only W_o c1 (+ its Y DMA) remains as tail.
  - Attention-phase filler accumulations use a dedicated 1-bank PSUM pool
    (upool shrunk to 3 bufs to free the bank) so they never collide with the
    psS QK^T/exp rotation.

mask is all-ones for this problem (spec fill="ones") and adds 0 to logits, so
it is not read. Compute dtype bf16 (inputs converted host-side), f32
accumulation; softmax without max-subtraction (logits are O(1) by
construction so exp never overflows).
"""

from contextlib import ExitStack

import numpy as np
import ml_dtypes

import concourse.bacc as bacc
import concourse.bass as bass
import concourse.mybir as mybir
import concourse.tile as tile
from concourse.bass_utils import run_bass_kernel_spmd

BF16 = mybir.dt.bfloat16
F32 = mybir.dt.float32
I32 = mybir.dt.int32
EXP = mybir.ActivationFunctionType.Exp
RECIP_MAGIC = 0x7EF311C3

B, S, D, H, K, V = 4, 2048, 768, 12, 64, 64
HPC = 6  # heads per core
CH = 512  # q chunk


def build_nc(S=S, D=D, HPC=HPC, K=K, CH=CH):
    """Build the per-core Bass program (SPMD: same program on all 8 cores)."""
    assert D % 128 == 0 and S % 128 == 0 and S % CH == 0 and K == 64
    DT = D // 128  # contraction tiles for projections
    KT = S // 128  # key-position tiles
    QC = S // CH  # q chunks
    NP = HPC // 2  # head pairs
    FW = HPC * K  # per-core projection feature width (384)
    assert FW // 128 == NP

    nc = bacc.Bacc("TRN2", target_bir_lowering=False, debug=False, num_devices=8)

    # all DRAM tensors are partition-major: [128, ...] with the row-block
    # index folded into the column axis (host packs them; see _prep_in_maps)
    xq = nc.declare_dram_parameter("xq", [128, DT * S], BF16, isOutput=False)
    xk = nc.declare_dram_parameter("xk", [128, DT * S], BF16, isOutput=False)
    xv = nc.declare_dram_parameter("xv", [128, DT * S], BF16, isOutput=False)
    wq = nc.declare_dram_parameter("wq", [128, DT * FW], BF16, isOutput=False)
    wk = nc.declare_dram_parameter("wk", [128, DT * FW], BF16, isOutput=False)
    wv = nc.declare_dram_parameter("wv", [128, DT * FW], BF16, isOutput=False)
    wo = nc.declare_dram_parameter("wo", [128, DT * D], BF16, isOutput=False)
    SOUT = S * HPC // H  # output rows produced by this core (1024)
    y = nc.declare_dram_parameter("y", [128, DT * SOUT], F32, isOutput=True)

    with tile.TileContext(nc) as tc, ExitStack() as ctx:
        xpool = ctx.enter_context(tc.tile_pool(name="xin", bufs=1))
        wpool = ctx.enter_context(tc.tile_pool(name="w", bufs=1))
        qkpool = ctx.enter_context(tc.tile_pool(name="qk", bufs=1))
        vpool = ctx.enter_context(tc.tile_pool(name="vhat", bufs=1))
        opool = ctx.enter_context(tc.tile_pool(name="ot", bufs=1))
        epool = ctx.enter_context(tc.tile_pool(name="exps", bufs=10))
        rpool = ctx.enter_context(tc.tile_pool(name="rec", bufs=6))
        unpool = ctx.enter_context(tc.tile_pool(name="un", bufs=4))
        ypool = ctx.enter_context(tc.tile_pool(name="yev", bufs=4))
        # PSUM budget (8 banks): psS 2x[128,1024] (4) + U 3x[128,512] (3)
        # + 1-bank filler accumulator apool (1) = 8. The prologue projections
        # (before the attention rotation starts) borrow psS "s" slots.
        pspool = ctx.enter_context(tc.tile_pool(name="ps", bufs=2, space="PSUM"))
        upool = ctx.enter_context(tc.tile_pool(name="us", bufs=3, space="PSUM"))
        apool = ctx.enter_context(tc.tile_pool(name="acc", bufs=1, space="PSUM"))

        def load_w(dram):
            """DRAM [128, n] (partition-major) -> SBUF tile [128, n]."""
            t = wpool.tile(
                [128, dram.shape[1]], BF16, tag=dram.name, name=dram.name + "_sb"
            )
            nc.sync.dma_start(t[:], dram[:, :])
            return t

        xq_sb = xpool.tile([128, DT * S], BF16, tag="xq")
        xk_sb = xpool.tile([128, DT * S], BF16, tag="xk")
        xv_sb = xpool.tile([128, DT * S], BF16, tag="xv")

        def load_x_chunk(t, dram, qc):
            # dram x layout: [128, (qc, n, m)] -- chunk qc is one contiguous
            # [128, DT*CH] block (6 KiB runs per partition)
            t3 = t[:].rearrange("p (n m) -> p n m", m=S)
            d4 = dram[:, :].rearrange("p (q n m) -> p q n m", q=QC, m=CH)
            nc.sync.dma_start(
                t3[:, :, qc * CH : qc * CH + CH],
                d4[:, qc],
            )

        # DMA issue order: xv first (V-proj is the bulk of prologue PE work
        # and runs shadowed by the rest of the load), then the MINIMAL
        # first-exp critical path wk+xk[c0]+wq+xq[c0] (attention kt 0-3 only
        # needs K-proj chunk 0), then the remaining xk/xq chunks in kt/qc
        # consumption order, wo last
        wv_sb = load_w(wv)
        for qc in range(QC):
            load_x_chunk(xv_sb, xv, qc)
        wk_sb = load_w(wk)
        load_x_chunk(xk_sb, xk, 0)
        wq_sb = load_w(wq)
        load_x_chunk(xq_sb, xq, 0)
        for qc in range(1, QC):
            load_x_chunk(xk_sb, xk, qc)
        for qc in range(1, QC):
            load_x_chunk(xq_sb, xq, qc)
        wo_sb = load_w(wo)

        # PE warm-up burst: dependency-free matmuls that run during the
        # initial DMA wait so the HAM clock gate is at 8/8 (2.4 GHz) when
        # the first projection matmuls issue
        wu = wpool.tile([128, 128], BF16, tag="warm", name="warm")
        nc.vector.memset(wu[:], 0.0)
        pswu = pspool.tile([128, 2 * CH], F32, tag="s", name="pswu")
        for _ in range(56):
            nc.tensor.matmul(pswu[:, 0:128], wu[:], wu[:], start=True, stop=True)

        qt_sb = [
            qkpool.tile([128, S], BF16, tag=f"qt{p}", name=f"qt{p}")
            for p in range(NP)
        ]
        kt_sb = [
            qkpool.tile([128, S], BF16, tag=f"kt{p}", name=f"kt{p}")
            for p in range(NP)
        ]
        # G^T tiles: row 64j+v, col c — g-th tile holds j in {2g, 2g+1}
        gt_sb = [
            opool.tile([128, SOUT], BF16, tag=f"gt{g}", name=f"gt{g}")
            for g in range(D // 128)
        ]
        # vhat[kt]: [128, HPC*128]; head h occupies cols [128h,128h+128) as
        # [V_h (64) | ones (64)] — the ones columns make the AV matmul also
        # produce sum(exp) replicated across partitions 64..127.
        vhat = [
            vpool.tile([128, HPC * 128], BF16, tag=f"vh{k}", name=f"vh{k}")
            for k in range(KT)
        ]

        # Projection / output-projection work is emitted as micro-units
        # (one matmul or one eviction per unit) so it can be interleaved
        # into the attention kt-loop: the PE queue is in-order, so coarse
        # blocks would head-of-line-block the QK^T matmuls that feed the
        # (bottleneck) ScalarE exp stream.
        def acc_group_units(mm_emit, fin_emit, pool_tag, pslice=None):
            st = {}
            pool = pspool if pool_tag == "s" else apool

            def mk(dt):
                def f():
                    if dt == 0:
                        width = 2 * CH if pool_tag == "s" else CH
                        ps = pool.tile([128, width], F32, tag=pool_tag, name="acc")
                        st["ps"] = ps[:, : pslice or CH]
                    mm_emit(st["ps"], dt)

                return f

            return [mk(dt) for dt in range(DT)] + [lambda: fin_emit(st["ps"])]

        def proj_chunk_units(w_sb, x_sb, dst, hp, qc, pool_tag):
            def mm(ps, dt):
                nc.tensor.matmul(
                    ps[:],
                    w_sb[:, dt * FW + hp * 128 : dt * FW + hp * 128 + 128],
                    x_sb[:, dt * S + qc * CH : dt * S + qc * CH + CH],
                    start=(dt == 0),
                    stop=(dt == DT - 1),
                )

            def fin(ps):
                nc.vector.tensor_copy(dst[:, qc * CH : qc * CH + CH], ps[:])

            return acc_group_units(mm, fin, pool_tag)

        def v_proj_units(kt, pool_tag="s"):
            # V projection for ALL heads at once (moving N=FW=384)
            def mm(pv, dt):
                nc.tensor.matmul(
                    pv[:],
                    xv_sb[:, dt * S + kt * 128 : dt * S + kt * 128 + 128],
                    wv_sb[:, dt * FW : dt * FW + FW],
                    start=(dt == 0),
                    stop=(dt == DT - 1),
                )

            def fin(pv):
                dst3 = vhat[kt][:].rearrange("p (h m) -> p h m", m=128)
                nc.vector.tensor_copy(
                    dst3[:, :, 0:64],
                    pv[:].rearrange("p (h m) -> p h m", m=64),
                )
                nc.vector.memset(dst3[:, :, 64:128], 1.0)

            return acc_group_units(mm, fin, pool_tag, pslice=FW)

        def v_proj_units_a(kt):
            return v_proj_units(kt, "a")

        GT = D // 128

        def _wo_dt_units(c0, cw, dt, pool_tag):
            # output projection for y rows [128*dt, 128*dt+128), cols [c0,
            # c0+cw): accumulate Y^T = sum_g W_o[g]^T @ G^T[g] over the GT
            # feature blocks
            def mm(py, g):
                nc.tensor.matmul(
                    py[:],
                    wo_sb[:, g * D + dt * 128 : g * D + dt * 128 + 128],
                    gt_sb[g][:, c0 : c0 + cw],
                    start=(g == 0),
                    stop=(g == GT - 1),
                )

            def fin(py):
                yt = ypool.tile([128, cw], F32, tag="yev", name="yt")
                nc.vector.tensor_copy(yt[:], py[:])
                nc.sync.dma_start(
                    y[:, dt * SOUT + c0 : dt * SOUT + c0 + cw],
                    yt[:],
                )

            st = {}
            pool = pspool if pool_tag == "s" else apool

            def mkmm(g):
                def f():
                    if g == 0:
                        width = 2 * CH if pool_tag == "s" else CH
                        ps = pool.tile([128, width], F32, tag=pool_tag, name="acc")
                        st["ps"] = ps[:, :cw]
                    mm(st["ps"], g)

                return f

            return [mkmm(g) for g in range(GT)] + [lambda: fin(st["ps"])]

        def wo_units(c0, cw, pool_tag):
            units = []
            for dt in range(DT):
                units += _wo_dt_units(c0, cw, dt, pool_tag)
            return units

        # ---- prologue: shadowed by the input DMAs ----
        # V proj kts 0-9 behind the xv loads (the rest go in as fillers:
        # V-proj's ~20us of PE issue time overshoots the xv DMA window)
        VPRE = 10
        for kt in range(VPRE):
            for un in v_proj_units(kt):
                un()
        # K proj pair0 chunk 0 + Q proj pair0 chunk 0: the minimal feed for
        # the first 4 attention kt slots
        for un in proj_chunk_units(wk_sb, xk_sb, kt_sb[0], 0, 0, "s"):
            un()
        for un in proj_chunk_units(wq_sb, xq_sb, qt_sb[0], 0, 0, "s"):
            un()

        # ---- filler queue: everything else, with CORRECTNESS deadlines.
        # Tile's hazard tracking is emission-ordered: a read emitted before
        # its producer sees garbage. Every filler unit therefore carries the
        # slot index it must be emitted by (the slot just before its first
        # reader's emission, which includes the one-slot QK^T lookahead);
        # the slot loop force-drains overdue units at slot start and
        # otherwise pops ahead at a steady rate. ----
        fillers = []  # (deadline_slot, unit), kept deadline-sorted

        def addf(deadline, units):
            fillers.extend((deadline, u) for u in units)

        for c in range(1, QC):
            addf(4 * c - 2, proj_chunk_units(wk_sb, xk_sb, kt_sb[0], 0, c, "a"))
        for kt in range(VPRE, KT):
            addf(kt, v_proj_units_a(kt))
        for c in range(1, QC):
            addf(16 * c - 2, proj_chunk_units(wq_sb, xq_sb, qt_sb[0], 0, c, "a"))
        for hp in (1, 2):
            for c in range(QC):
                addf(
                    64 * hp - 2,
                    proj_chunk_units(wk_sb, xk_sb, kt_sb[hp], hp, c, "a"),
                )
            for c in range(QC):
                addf(
                    64 * hp - 2,
                    proj_chunk_units(wq_sb, xq_sb, qt_sb[hp], hp, c, "a"),
                )
        fillers.sort(key=lambda du: du[0])

        late_fillers = []  # W_o chunks: gated on the gt scatters they read

        def pop_fillers(n):
            for _ in range(n):
                if fillers:
                    fillers.pop(0)[1]()
                elif late_fillers:
                    late_fillers.pop()()

        def emit_qkt(hp, qc, kt):
            psS = pspool.tile([128, 2 * CH], F32, tag="s", name="psS")
            # row-packed pair: head A rows 0-63, head B rows 64-127; the two
            # matmuls land on disjoint PE row-groups and run concurrently
            nc.tensor.matmul(
                psS[:, 0:CH],
                kt_sb[hp][0:64, kt * 128 : kt * 128 + 128],
                qt_sb[hp][0:64, qc * CH : qc * CH + CH],
                start=True,
                stop=True,
            )
            nc.tensor.matmul(
                psS[:, CH : 2 * CH],
                kt_sb[hp][64:128, kt * 128 : kt * 128 + 128],
                qt_sb[hp][64:128, qc * CH : qc * CH + CH],
                start=True,
                stop=True,
            )
            return psS

        scat_q = []  # deferred gt-scatter copies (DVE), drained ~3/slot

        def emit_normalize(u_a, u_b, hp, qc, defer=True):
            # Newton reciprocal of the replicated exp-sums in rows 64..127
            # (magic seed + 2 NR passes; w holds -1/l at ~1e-5 rel err).
            # The two heads' chains are interleaved per-op so the DVE
            # pipeline stays full (the chain is serially dependent per head)
            pair = ((u_a, 2 * hp), (u_b, 2 * hp + 1))
            rs, ts, ws, uns = [], [], [], []
            for u, hl in pair:
                r = rpool.tile([64, CH], F32, tag="rec", name="r")
                nc.vector.tensor_scalar(
                    r[:].bitcast(I32), u[64:128, :].bitcast(I32),
                    RECIP_MAGIC, -1,
                    mybir.AluOpType.subtract, mybir.AluOpType.mult,
                )
                rs.append(r)
            for (u, hl), r in zip(pair, rs):
                t = rpool.tile([64, CH], F32, tag="rec", name="t")
                nc.vector.tensor_mul(t[:], u[64:128, :], r[:])
                ts.append(t)
            for r, t in zip(rs, ts):
                w = rpool.tile([64, CH], F32, tag="rec", name="w")
                nc.vector.scalar_tensor_tensor(
                    w[:], t[:], 2.0, r[:],
                    mybir.AluOpType.subtract, mybir.AluOpType.mult,
                )
                ws.append(w)
            for (u, hl), w in zip(pair, ws):
                # stage the normalized output to SBUF in ONE op so the PSUM
                # u slot frees fast (u pool is only 3 bufs deep); the gt
                # scatter then reads the staging tile off the critical path
                un = unpool.tile([64, CH], F32, tag="un", name="un")
                nc.vector.scalar_tensor_tensor(
                    un[:], u[0:64, :], -1.0, w[:],
                    mybir.AluOpType.mult, mybir.AluOpType.mult,
                )
                uns.append(un)

            # scatter: Ot[v, s] -> G^T[64j+v, c] with j=(S*hl+s)%H,
            # c=(S*hl+s)//H; strided in s (step H). Deferred into the next
            # qc's slots so the 24-copy DVE burst never backs up the filler
            # evictions (whose PSUM WAR would stall the PE queue)
            for (u, hl), un in zip(pair, uns):
                cq0 = qc * CH
                for j in range(H):
                    s0 = (j - S * hl) % H
                    m0 = max(0, -(-(cq0 - s0) // H))
                    s_st = s0 + H * m0
                    if s_st >= cq0 + CH:
                        continue
                    count = (cq0 + CH - 1 - s_st) // H + 1
                    o = s_st - cq0
                    c_st = (S * hl + s_st) // H
                    sl = slice(o, o + H * (count - 1) + 1, H)

                    def cp(un=un, j=j, c_st=c_st, count=count, sl=sl):
                        nc.vector.tensor_copy(
                            gt_sb[j // 2][
                                64 * (j % 2) : 64 * (j % 2) + 64,
                                c_st : c_st + count,
                            ],
                            un[:, sl],
                        )

                    if defer:
                        scat_q.append(cp)
                    else:
                        cp()

        # ---- attention: flat kt stream with one-slot QK^T lookahead so the
        # next qc's logits are already in flight when a qc ends (keeps the
        # exp stream gap-free across qc boundaries) ----
        slots = [
            (hp, qc, kt)
            for hp in range(NP)
            for qc in range(QC)
            for kt in range(KT)
        ]
        # W_o 256-col chunks become available as the gt bands they read
        # finish scattering: chunk c is gated on (pair, qc) per the scramble
        # geometry; (slot_index -> chunk) emission gates (one qc of margin
        # for the deferred scatters):
        WCW = 256
        wo_gate = {
            1 * QC * KT + 1 * KT: 0,  # [0:256)    during pair1 (needs pair0)
            2 * QC * KT + 1 * KT: 1,  # [256:512)  during pair2 (needs p1)
            2 * QC * KT + 3 * KT: 2,  # [512:768)  during p2 qc3 (hl4 qc0-1)
        }
        psS_cur = emit_qkt(*slots[0])
        u_a = u_b = None
        for g, (hp, qc, kt) in enumerate(slots):
            # correctness: emit every filler whose first reader sits in this
            # slot (or the next slot's QK^T) before any attention unit
            while fillers and fillers[0][0] <= g:
                fillers.pop(0)[1]()
            if g in wo_gate:
                # prepend reversed (consumed via pop() from the end)
                late_fillers[:0] = reversed(wo_units(wo_gate[g] * WCW, WCW, "a"))
            if kt == 0:
                u_a = upool.tile([128, CH], F32, tag="u", name="ua")
                u_b = upool.tile([128, CH], F32, tag="u", name="ub")
            es = epool.tile([128, 2 * CH], BF16, tag="es", name="es")
            nc.scalar.activation(es[:], psS_cur[:], EXP)
            if g + 1 < len(slots):
                psS_cur = emit_qkt(*slots[g + 1])
            nc.tensor.matmul(
                u_a[:],
                vhat[kt][:, 256 * hp : 256 * hp + 128],
                es[:, 0:CH],
                start=(kt == 0),
                stop=(kt == KT - 1),
            )
            nc.tensor.matmul(
                u_b[:],
                vhat[kt][:, 256 * hp + 128 : 256 * hp + 256],
                es[:, CH : 2 * CH],
                start=(kt == 0),
                stop=(kt == KT - 1),
            )
            if kt == KT - 1:
                emit_normalize(u_a, u_b, hp, qc, defer=(g + 1 < len(slots)))
            # drain deferred scatter copies (DVE; no PE cost)
            for _ in range(3):
                if scat_q:
                    scat_q.pop(0)()
            # PE fillers: stay ahead in the early (V-proj/K-proj) era, and
            # throttle at qc boundaries so PSUM-WAR chains never sit in
            # front of the next qc's QK^T
            if kt in (0, KT - 1):
                pop_fillers(1)
            elif fillers:
                pop_fillers(4 if g < 16 else 2)
            else:
                pop_fillers(3)  # wo late-filler era: M=256 units are cheap

        # tail: leftover scatters + fillers, then W_o [768:1024) on the
        # roomy "s" pool (psS is free now -> no 1-buf WAR chain)
        while scat_q:
            scat_q.pop(0)()
        while fillers or late_fillers:
            pop_fillers(1)
        for un in wo_units(3 * WCW, WCW, "s"):
            un()

    nc.compile()
    return nc


_NC_CACHE = None


def _get_nc():
    global _NC_CACHE
    if _NC_CACHE is None:
        _NC_CACHE = build_nc()
    return _NC_CACHE


def _pm_x(a):
    # [S, D] f32 -> X^T partition-major [128, QC, DT, CH] -> [128, DT*S]
    bf = ml_dtypes.bfloat16
    QC = S // CH
    DT = D // 128
    t = a.T.astype(bf)  # [D, S]
    return (
        t.reshape(DT, 128, QC, CH).transpose(1, 2, 0, 3).reshape(128, DT * S)
    )


def _pm_w(w):
    # [D, F] -> partition-major [128, DT*F]
    bf = ml_dtypes.bfloat16
    DT = D // 128
    F = w.shape[1]
    return w.astype(bf).reshape(DT, 128, F).transpose(1, 0, 2).reshape(128, DT * F)


def _prep_in_maps(queries, keys, values, W_q, W_k, W_v, W_o):
    scale = np.float32(1.0 / np.sqrt(K))
    in_maps = []
    xq_pm = [_pm_x(queries[b]) for b in range(B)]
    xk_pm = [_pm_x(keys[b]) for b in range(B)]
    xv_pm = [_pm_x(values[b]) for b in range(B)]
    wo_pm = _pm_w(W_o)  # full W_o: the raw-reshape scramble touches all rows
    for core in range(8):
        b, hg = divmod(core, 2)
        h0 = hg * HPC
        wq_c = (W_q[h0 : h0 + HPC] * scale).transpose(1, 0, 2).reshape(D, HPC * K)
        wk_c = W_k[h0 : h0 + HPC].transpose(1, 0, 2).reshape(D, HPC * K)
        wv_c = W_v[h0 : h0 + HPC].transpose(1, 0, 2).reshape(D, HPC * V)
        in_maps.append(
            {
                "xq": xq_pm[b],
                "xk": xk_pm[b],
                "xv": xv_pm[b],
                "wq": _pm_w(wq_c),
                "wk": _pm_w(wk_c),
                "wv": _pm_w(wv_c),
                "wo": wo_pm,
            }
        )
    return in_maps


def run(inputs, trace=False, **spmd_kwargs):
    """Run on 8 cores; returns (full_output [B,S,D] f32, BassKernelResults)."""
    queries = np.asarray(inputs["queries"], np.float32)
    keys = np.asarray(inputs["keys"], np.float32)
    values = np.asarray(inputs["values"], np.float32)
    W_q = np.asarray(inputs["W_q"], np.float32)
    W_k = np.asarray(inputs["W_k"], np.float32)
    W_v = np.asarray(inputs["W_v"], np.float32)
    W_o = np.asarray(inputs["W_o"], np.float32)

    nc = _get_nc()
    in_maps = _prep_in_maps(queries, keys, values, W_q, W_k, W_v, W_o)
    res = run_bass_kernel_spmd(
        nc, in_maps, core_ids=list(range(8)), trace=trace, **spmd_kwargs
    )
    out = np.empty((B, S, D), np.float32)
    half = S * HPC // H  # 1024 output rows per head-group core
    DT = D // 128
    for b in range(B):
        for hg in range(2):
            y_pm = res.results[2 * b + hg]["y"]  # [128, DT*half]
            yt = y_pm.reshape(128, DT, half).transpose(1, 0, 2).reshape(D, half)
            out[b, hg * half : (hg + 1) * half] = yt.T
    return out, res


def kernel(**inputs) -> np.ndarray:
    out, _ = run(inputs, trace=False)
    return out


# revision 23
# speedup vs baseline: 1.0010x; 1.0010x over previous
"""Trainium2 8-core kernel for nn_MultiHeadAttention_83408264889124.

Full inputs in, full output out. Sharding: batch (4) x head-group (2) grid
over 8 NeuronCores — each core computes one batch with 6 of the 12 heads and
produces a partial Y^T = sum_h W_o[h]^T @ O_h^T; the host adds the two
head-group partials per batch (the "all-reduce" of the TP split) and
transposes back. All device work is in transposed layouts so no on-device
transposes are needed:

  Qt = (wq/sqrt(K))^T X^T, Kt = wk^T X^T          [K, S] per head
  St[k,q] = sum_d Kt[d,k] Qt[d,q]                  (2-head row-packed matmuls)
  E = exp(St)  (ScalarE, PSUM->SBUF bf16)
  AV with stationary [V_h | ones*64]: U[0:64] = V^T E, U[64:128] = colsum(E)
  Ot = U[0:64] * recip(U[64:128])                  (VectorE)

The reference does a RAW reshape [B,H,S,V] -> [B,S,H*V] (no transpose) before
W_o, which scrambles (head, seq): with t = S*h + s, output row s' = t//H gets
feature block j = t%H from head h, position s. Because S*HPC/H = 1024 exactly,
each head-group core produces a clean half of the output rows, and the scatter
indices depend only on the LOCAL head index - so one SPMD program works for
all cores. The output projection is Y^T = W_o^T @ G^T with the FULL W_o.

Schedule (the perf-critical part): the ScalarE exp stream is the bottleneck
resource (192 ACTIVATEs x ~1.1us with zero slack). The kernel therefore
minimizes time outside the exp-saturated window:
  - DMA order: wk, xk, wv, xv, wq, xq[c0], xq[c1:], wo; all DRAM tensors are
    host-packed partition-major ([128, ...] with large contiguous runs) so
    DMA runs at full rate.
  - Prologue (overlapped with DMA): PE warm-up burst, K-proj pair0 chunk-by-
    chunk behind the xk chunk DMAs, V-proj all kt behind xv, Q-proj pair0
    chunk0. First exp issues ~27us in.
  - All remaining projections (K/Q pairs 1-2, Q pair0 chunks 1-3) and the
    first output-projection column chunk (W_o c0 depends only on heads 0-2 =
    pairs 0,1) are emitted as single-matmul micro-units interleaved ~2 per kt
    iteration into the attention loops, filling the PE slack under the exp
    stream. Only W_o c1 (+ its Y DMA) remains as tail.
  - Attention-phase filler accumulations use a dedicated 1-bank PSUM pool
    (upool shrunk to 3 bufs to free the bank) so they never collide with the
    psS QK^T/exp rotation.

mask is all-ones for this problem (spec fill="ones") and adds 0 to logits, so
it is not read. Compute dtype bf16 (inputs converted host-side), f32
accumulation; softmax without max-subtraction (logits are O(1) by
construction so exp never overflows).
"""

from contextlib import ExitStack

import numpy as np
import ml_dtypes

import concourse.bacc as bacc
import concourse.bass as bass
import concourse.mybir as mybir
import concourse.tile as tile
from concourse.bass_utils import run_bass_kernel_spmd

BF16 = mybir.dt.bfloat16
F32 = mybir.dt.float32
I32 = mybir.dt.int32
EXP = mybir.ActivationFunctionType.Exp
RECIP_MAGIC = 0x7EF311C3

B, S, D, H, K, V = 4, 2048, 768, 12, 64, 64
HPC = 6  # heads per core
CH = 512  # q chunk


def build_nc(S=S, D=D, HPC=HPC, K=K, CH=CH):
    """Build the per-core Bass program (SPMD: same program on all 8 cores)."""
    assert D % 128 == 0 and S % 128 == 0 and S % CH == 0 and K == 64
    DT = D // 128  # contraction tiles for projections
    KT = S // 128  # key-position tiles
    QC = S // CH  # q chunks
    NP = HPC // 2  # head pairs
    FW = HPC * K  # per-core projection feature width (384)
    assert FW // 128 == NP

    nc = bacc.Bacc("TRN2", target_bir_lowering=False, debug=False, num_devices=8)

    # all DRAM tensors are partition-major: [128, ...] with the row-block
    # index folded into the column axis (host packs them; see _prep_in_maps)
    xq = nc.declare_dram_parameter("xq", [128, DT * S], BF16, isOutput=False)
    xk = nc.declare_dram_parameter("xk", [128, DT * S], BF16, isOutput=False)
    xv = nc.declare_dram_parameter("xv", [128, DT * S], BF16, isOutput=False)
    wq = nc.declare_dram_parameter("wq", [128, DT * FW], BF16, isOutput=False)
    wk = nc.declare_dram_parameter("wk", [128, DT * FW], BF16, isOutput=False)
    wv = nc.declare_dram_parameter("wv", [128, DT * FW], BF16, isOutput=False)
    wo = nc.declare_dram_parameter("wo", [128, DT * D], BF16, isOutput=False)
    SOUT = S * HPC // H  # output rows produced by this core (1024)
    y = nc.declare_dram_parameter("y", [128, DT * SOUT], F32, isOutput=True)

    with tile.TileContext(nc) as tc, ExitStack() as ctx:
        xpool = ctx.enter_context(tc.tile_pool(name="xin", bufs=1))
        wpool = ctx.enter_context(tc.tile_pool(name="w", bufs=1))
        qkpool = ctx.enter_context(tc.tile_pool(name="qk", bufs=1))
        vpool = ctx.enter_context(tc.tile_pool(name="vhat", bufs=1))
        opool = ctx.enter_context(tc.tile_pool(name="ot", bufs=1))
        epool = ctx.enter_context(tc.tile_pool(name="exps", bufs=10))
        rpool = ctx.enter_context(tc.tile_pool(name="rec", bufs=6))
        unpool = ctx.enter_context(tc.tile_pool(name="un", bufs=4))
        ypool = ctx.enter_context(tc.tile_pool(name="yev", bufs=4))
        # PSUM budget (8 banks): psS 2x[128,1024] (4) + U 3x[128,512] (3)
        # + 1-bank filler accumulator apool (1) = 8. The prologue projections
        # (before the attention rotation starts) borrow psS "s" slots.
        pspool = ctx.enter_context(tc.tile_pool(name="ps", bufs=2, space="PSUM"))
        upool = ctx.enter_context(tc.tile_pool(name="us", bufs=3, space="PSUM"))
        apool = ctx.enter_context(tc.tile_pool(name="acc", bufs=1, space="PSUM"))

        def load_w(dram):
            """DRAM [128, n] (partition-major) -> SBUF tile [128, n]."""
            t = wpool.tile(
                [128, dram.shape[1]], BF16, tag=dram.name, name=dram.name + "_sb"
            )
            nc.sync.dma_start(t[:], dram[:, :])
            return t

        xq_sb = xpool.tile([128, DT * S], BF16, tag="xq")
        xk_sb = xpool.tile([128, DT * S], BF16, tag="xk")
        xv_sb = xpool.tile([128, DT * S], BF16, tag="xv")

        def load_x_chunk(t, dram, qc):
            # dram x layout: [128, (qc, n, m)] -- chunk qc is one contiguous
            # [128, DT*CH] block (6 KiB runs per partition)
            t3 = t[:].rearrange("p (n m) -> p n m", m=S)
            d4 = dram[:, :].rearrange("p (q n m) -> p q n m", q=QC, m=CH)
            nc.sync.dma_start(
                t3[:, :, qc * CH : qc * CH + CH],
                d4[:, qc],
            )

        # DMA issue order: xv first (V-proj is the bulk of prologue PE work
        # and runs shadowed by the rest of the load), then the MINIMAL
        # first-exp critical path wk+xk[c0]+wq+xq[c0] (attention kt 0-3 only
        # needs K-proj chunk 0), then the remaining xk/xq chunks in kt/qc
        # consumption order, wo last
        wv_sb = load_w(wv)
        for qc in range(QC):
            load_x_chunk(xv_sb, xv, qc)
        wk_sb = load_w(wk)
        load_x_chunk(xk_sb, xk, 0)
        wq_sb = load_w(wq)
        load_x_chunk(xq_sb, xq, 0)
        for qc in range(1, QC):
            load_x_chunk(xk_sb, xk, qc)
        for qc in range(1, QC):
            load_x_chunk(xq_sb, xq, qc)
        wo_sb = load_w(wo)

        # PE warm-up burst: dependency-free matmuls that run during the
        # initial DMA wait so the HAM clock gate is at 8/8 (2.4 GHz) when
        # the first projection matmuls issue
        wu = wpool.tile([128, 128], BF16, tag="warm", name="warm")
        nc.vector.memset(wu[:], 0.0)
        pswu = pspool.tile([128, 2 * CH], F32, tag="s", name="pswu")
        for _ in range(56):
            nc.tensor.matmul(pswu[:, 0:128], wu[:], wu[:], start=True, stop=True)
        # dummy exp during the DMA wait: pulls the ~2.7us ACT table load
        # (PSEUDO_LOAD_ACT_FUNC_SET rides on the first ACTIVATE) off the
        # first real exp's critical path
        edum = wpool.tile([128, 16], BF16, tag="edum", name="edum")
        nc.scalar.activation(edum[:], pswu[:, 0:16], EXP)

        qt_sb = [
            qkpool.tile([128, S], BF16, tag=f"qt{p}", name=f"qt{p}")
            for p in range(NP)
        ]
        kt_sb = [
            qkpool.tile([128, S], BF16, tag=f"kt{p}", name=f"kt{p}")
            for p in range(NP)
        ]
        # G^T tiles: row 64j+v, col c — g-th tile holds j in {2g, 2g+1}
        gt_sb = [
            opool.tile([128, SOUT], BF16, tag=f"gt{g}", name=f"gt{g}")
            for g in range(D // 128)
        ]
        # vhat[kt]: [128, HPC*128]; head h occupies cols [128h,128h+128) as
        # [V_h (64) | ones (64)] — the ones columns make the AV matmul also
        # produce sum(exp) replicated across partitions 64..127.
        vhat = [
            vpool.tile([128, HPC * 128], BF16, tag=f"vh{k}", name=f"vh{k}")
            for k in range(KT)
        ]

        # Projection / output-projection work is emitted as micro-units
        # (one matmul or one eviction per unit) so it can be interleaved
        # into the attention kt-loop: the PE queue is in-order, so coarse
        # blocks would head-of-line-block the QK^T matmuls that feed the
        # (bottleneck) ScalarE exp stream.
        def acc_group_units(mm_emit, fin_emit, pool_tag, pslice=None):
            st = {}
            pool = pspool if pool_tag == "s" else apool

            def mk(dt):
                def f():
                    if dt == 0:
                        width = 2 * CH if pool_tag == "s" else CH
                        ps = pool.tile([128, width], F32, tag=pool_tag, name="acc")
                        st["ps"] = ps[:, : pslice or CH]
                    mm_emit(st["ps"], dt)

                return f

            return [mk(dt) for dt in range(DT)] + [lambda: fin_emit(st["ps"])]

        def proj_chunk_units(w_sb, x_sb, dst, hp, qc, pool_tag):
            def mm(ps, dt):
                nc.tensor.matmul(
                    ps[:],
                    w_sb[:, dt * FW + hp * 128 : dt * FW + hp * 128 + 128],
                    x_sb[:, dt * S + qc * CH : dt * S + qc * CH + CH],
                    start=(dt == 0),
                    stop=(dt == DT - 1),
                )

            def fin(ps):
                nc.vector.tensor_copy(dst[:, qc * CH : qc * CH + CH], ps[:])

            return acc_group_units(mm, fin, pool_tag)

        def v_proj_units(kt, pool_tag="s"):
            # V projection for ALL heads at once (moving N=FW=384)
            def mm(pv, dt):
                nc.tensor.matmul(
                    pv[:],
                    xv_sb[:, dt * S + kt * 128 : dt * S + kt * 128 + 128],
                    wv_sb[:, dt * FW : dt * FW + FW],
                    start=(dt == 0),
                    stop=(dt == DT - 1),
                )

            def fin(pv):
                dst3 = vhat[kt][:].rearrange("p (h m) -> p h m", m=128)
                nc.vector.tensor_copy(
                    dst3[:, :, 0:64],
                    pv[:].rearrange("p (h m) -> p h m", m=64),
                )
                nc.vector.memset(dst3[:, :, 64:128], 1.0)

            return acc_group_units(mm, fin, pool_tag, pslice=FW)

        def v_proj_units_a(kt):
            return v_proj_units(kt, "a")

        GT = D // 128

        def _wo_dt_units(c0, cw, dt, pool_tag):
            # output projection for y rows [128*dt, 128*dt+128), cols [c0,
            # c0+cw): accumulate Y^T = sum_g W_o[g]^T @ G^T[g] over the GT
            # feature blocks
            def mm(py, g):
                nc.tensor.matmul(
                    py[:],
                    wo_sb[:, g * D + dt * 128 : g * D + dt * 128 + 128],
                    gt_sb[g][:, c0 : c0 + cw],
                    start=(g == 0),
                    stop=(g == GT - 1),
                )

            def fin(py):
                yt = ypool.tile([128, cw], F32, tag="yev", name="yt")
                nc.vector.tensor_copy(yt[:], py[:])
                nc.sync.dma_start(
                    y[:, dt * SOUT + c0 : dt * SOUT + c0 + cw],
                    yt[:],
                )

            st = {}
            pool = pspool if pool_tag == "s" else apool

            def mkmm(g):
                def f():
                    if g == 0:
                        width = 2 * CH if pool_tag == "s" else CH
                        ps = pool.tile([128, width], F32, tag=pool_tag, name="acc")
                        st["ps"] = ps[:, :cw]
                    mm(st["ps"], g)

                return f

            return [mkmm(g) for g in range(GT)] + [lambda: fin(st["ps"])]

        def wo_units(c0, cw, pool_tag):
            units = []
            for dt in range(DT):
                units += _wo_dt_units(c0, cw, dt, pool_tag)
            return units

        # ---- prologue: shadowed by the input DMAs ----
        # V proj kts 0-9 behind the xv loads (the rest go in as fillers:
        # V-proj's ~20us of PE issue time overshoots the xv DMA window)
        VPRE = 10
        for kt in range(VPRE):
            for un in v_proj_units(kt):
                un()
        # K proj pair0 chunk 0 + Q proj pair0 chunk 0: the minimal feed for
        # the first 4 attention kt slots
        for un in proj_chunk_units(wk_sb, xk_sb, kt_sb[0], 0, 0, "s"):
            un()
        for un in proj_chunk_units(wq_sb, xq_sb, qt_sb[0], 0, 0, "s"):
            un()

        # ---- filler queue: everything else, with CORRECTNESS deadlines.
        # Tile's hazard tracking is emission-ordered: a read emitted before
        # its producer sees garbage. Every filler unit therefore carries the
        # slot index it must be emitted by (the slot just before its first
        # reader's emission, which includes the one-slot QK^T lookahead);
        # the slot loop force-drains overdue units at slot start and
        # otherwise pops ahead at a steady rate. ----
        fillers = []  # (deadline_slot, unit), kept deadline-sorted

        def addf(deadline, units):
            fillers.extend((deadline, u) for u in units)

        for c in range(1, QC):
            addf(4 * c - 2, proj_chunk_units(wk_sb, xk_sb, kt_sb[0], 0, c, "a"))
        for kt in range(VPRE, KT):
            addf(kt, v_proj_units_a(kt))
        for c in range(1, QC):
            addf(16 * c - 2, proj_chunk_units(wq_sb, xq_sb, qt_sb[0], 0, c, "a"))
        # next pairs' K/Q proj: deadline-spread across the preceding pair's
        # slots so the force-drain never dumps a big burst in one slot
        for hp in (1, 2):
            base = 64 * (hp - 1) + 20
            for i, c in enumerate(range(QC)):
                addf(
                    base + 5 * i,
                    proj_chunk_units(wk_sb, xk_sb, kt_sb[hp], hp, c, "a"),
                )
            for i, c in enumerate(range(QC)):
                addf(
                    base + 5 * (4 + i),
                    proj_chunk_units(wq_sb, xq_sb, qt_sb[hp], hp, c, "a"),
                )
        fillers.sort(key=lambda du: du[0])

        late_fillers = []  # W_o chunks: gated on the gt scatters they read

        def pop_fillers(n):
            for _ in range(n):
                if fillers:
                    fillers.pop(0)[1]()
                elif late_fillers:
                    late_fillers.pop()()

        def emit_qkt(hp, qc, kt):
            psS = pspool.tile([128, 2 * CH], F32, tag="s", name="psS")
            # row-packed pair: head A rows 0-63, head B rows 64-127; the two
            # matmuls land on disjoint PE row-groups and run concurrently
            nc.tensor.matmul(
                psS[:, 0:CH],
                kt_sb[hp][0:64, kt * 128 : kt * 128 + 128],
                qt_sb[hp][0:64, qc * CH : qc * CH + CH],
                start=True,
                stop=True,
            )
            nc.tensor.matmul(
                psS[:, CH : 2 * CH],
                kt_sb[hp][64:128, kt * 128 : kt * 128 + 128],
                qt_sb[hp][64:128, qc * CH : qc * CH + CH],
                start=True,
                stop=True,
            )
            return psS

        scat_q = []  # deferred gt-scatter copies (DVE), drained ~3/slot

        def emit_normalize(u_a, u_b, hp, qc, defer=True):
            # Newton reciprocal of the replicated exp-sums in rows 64..127
            # (magic seed + 2 NR passes; w holds -1/l at ~1e-5 rel err).
            # The two heads' chains are interleaved per-op so the DVE
            # pipeline stays full (the chain is serially dependent per head)
            pair = ((u_a, 2 * hp), (u_b, 2 * hp + 1))
            rs, ts, ws, uns = [], [], [], []
            for u, hl in pair:
                r = rpool.tile([64, CH], F32, tag="rec", name="r")
                nc.vector.tensor_scalar(
                    r[:].bitcast(I32), u[64:128, :].bitcast(I32),
                    RECIP_MAGIC, -1,
                    mybir.AluOpType.subtract, mybir.AluOpType.mult,
                )
                rs.append(r)
            for (u, hl), r in zip(pair, rs):
                t = rpool.tile([64, CH], F32, tag="rec", name="t")
                nc.vector.tensor_mul(t[:], u[64:128, :], r[:])
                ts.append(t)
            for r, t in zip(rs, ts):
                w = rpool.tile([64, CH], F32, tag="rec", name="w")
                nc.vector.scalar_tensor_tensor(
                    w[:], t[:], 2.0, r[:],
                    mybir.AluOpType.subtract, mybir.AluOpType.mult,
                )
                ws.append(w)
            for (u, hl), w in zip(pair, ws):
                # stage the normalized output to SBUF in ONE op so the PSUM
                # u slot frees fast (u pool is only 3 bufs deep); the gt
                # scatter then reads the staging tile off the critical path
                un = unpool.tile([64, CH], F32, tag="un", name="un")
                nc.vector.scalar_tensor_tensor(
                    un[:], u[0:64, :], -1.0, w[:],
                    mybir.AluOpType.mult, mybir.AluOpType.mult,
                )
                uns.append(un)

            # scatter: Ot[v, s] -> G^T[64j+v, c] with j=(S*hl+s)%H,
            # c=(S*hl+s)//H; strided in s (step H). Deferred into the next
            # qc's slots so the 24-copy DVE burst never backs up the filler
            # evictions (whose PSUM WAR would stall the PE queue)
            for (u, hl), un in zip(pair, uns):
                cq0 = qc * CH
                for j in range(H):
                    s0 = (j - S * hl) % H
                    m0 = max(0, -(-(cq0 - s0) // H))
                    s_st = s0 + H * m0
                    if s_st >= cq0 + CH:
                        continue
                    count = (cq0 + CH - 1 - s_st) // H + 1
                    o = s_st - cq0
                    c_st = (S * hl + s_st) // H
                    sl = slice(o, o + H * (count - 1) + 1, H)

                    def cp(un=un, j=j, c_st=c_st, count=count, sl=sl):
                        nc.vector.tensor_copy(
                            gt_sb[j // 2][
                                64 * (j % 2) : 64 * (j % 2) + 64,
                                c_st : c_st + count,
                            ],
                            un[:, sl],
                        )

                    if defer:
                        scat_q.append(cp)
                    else:
                        cp()

        # ---- attention: flat kt stream with one-slot QK^T lookahead so the
        # next qc's logits are already in flight when a qc ends (keeps the
        # exp stream gap-free across qc boundaries) ----
        slots = [
            (hp, qc, kt)
            for hp in range(NP)
            for qc in range(QC)
            for kt in range(KT)
        ]
        # W_o 256-col chunks become available as the gt bands they read
        # finish scattering: chunk c is gated on (pair, qc) per the scramble
        # geometry; (slot_index -> chunk) emission gates (one qc of margin
        # for the deferred scatters):
        WCW = 256
        wo_gate = {
            1 * QC * KT + 1 * KT: 0,  # [0:256)    during pair1 (needs pair0)
            2 * QC * KT + 1 * KT: 1,  # [256:512)  during pair2 (needs p1)
            2 * QC * KT + 3 * KT: 2,  # [512:768)  during p2 qc3 (hl4 qc0-1)
        }
        psS_cur = emit_qkt(*slots[0])
        u_a = u_b = None
        for g, (hp, qc, kt) in enumerate(slots):
            # correctness: emit every filler whose first reader sits in this
            # slot (or the next slot's QK^T) before any attention unit
            while fillers and fillers[0][0] <= g:
                fillers.pop(0)[1]()
            if g in wo_gate:
                # prepend reversed (consumed via pop() from the end)
                late_fillers[:0] = reversed(wo_units(wo_gate[g] * WCW, WCW, "a"))
            if kt == 0:
                u_a = upool.tile([128, CH], F32, tag="u", name="ua")
                u_b = upool.tile([128, CH], F32, tag="u", name="ub")
            es = epool.tile([128, 2 * CH], BF16, tag="es", name="es")
            nc.scalar.activation(es[:], psS_cur[:], EXP)
            if g + 1 < len(slots):
                psS_cur = emit_qkt(*slots[g + 1])
            nc.tensor.matmul(
                u_a[:],
                vhat[kt][:, 256 * hp : 256 * hp + 128],
                es[:, 0:CH],
                start=(kt == 0),
                stop=(kt == KT - 1),
            )
            nc.tensor.matmul(
                u_b[:],
                vhat[kt][:, 256 * hp + 128 : 256 * hp + 256],
                es[:, CH : 2 * CH],
                start=(kt == 0),
                stop=(kt == KT - 1),
            )
            if kt == KT - 1:
                emit_normalize(u_a, u_b, hp, qc, defer=(g + 1 < len(slots)))
            # drain deferred scatter copies (DVE; no PE cost)
            for _ in range(3):
                if scat_q:
                    scat_q.pop(0)()
            # PE fillers: stay ahead in the early (V-proj/K-proj) era, and
            # throttle at qc boundaries so PSUM-WAR chains never sit in
            # front of the next qc's QK^T
            if kt in (0, KT - 1):
                pop_fillers(1)
            elif fillers:
                pop_fillers(4 if g < 16 else 2)
            else:
                pop_fillers(3)  # wo late-filler era: M=256 units are cheap

        # tail: leftover scatters + fillers, then W_o [768:1024) on the
        # roomy "s" pool (psS is free now -> no 1-buf WAR chain)
        while scat_q:
            scat_q.pop(0)()
        while fillers or late_fillers:
            pop_fillers(1)
        for un in wo_units(3 * WCW, WCW, "s"):
            un()

    nc.compile()
    return nc


_NC_CACHE = None


def _get_nc():
    global _NC_CACHE
    if _NC_CACHE is None:
        _NC_CACHE = build_nc()
    return _NC_CACHE


def _pm_x(a):
    # [S, D] f32 -> X^T partition-major [128, QC, DT, CH] -> [128, DT*S]
    bf = ml_dtypes.bfloat16
    QC = S // CH
    DT = D // 128
    t = a.T.astype(bf)  # [D, S]
    return (
        t.reshape(DT, 128, QC, CH).transpose(1, 2, 0, 3).reshape(128, DT * S)
    )


def _pm_w(w):
    # [D, F] -> partition-major [128, DT*F]
    bf = ml_dtypes.bfloat16
    DT = D // 128
    F = w.shape[1]
    return w.astype(bf).reshape(DT, 128, F).transpose(1, 0, 2).reshape(128, DT * F)


def _prep_in_maps(queries, keys, values, W_q, W_k, W_v, W_o):
    scale = np.float32(1.0 / np.sqrt(K))
    in_maps = []
    xq_pm = [_pm_x(queries[b]) for b in range(B)]
    xk_pm = [_pm_x(keys[b]) for b in range(B)]
    xv_pm = [_pm_x(values[b]) for b in range(B)]
    wo_pm = _pm_w(W_o)  # full W_o: the raw-reshape scramble touches all rows
    for core in range(8):
        b, hg = divmod(core, 2)
        h0 = hg * HPC
        wq_c = (W_q[h0 : h0 + HPC] * scale).transpose(1, 0, 2).reshape(D, HPC * K)
        wk_c = W_k[h0 : h0 + HPC].transpose(1, 0, 2).reshape(D, HPC * K)
        wv_c = W_v[h0 : h0 + HPC].transpose(1, 0, 2).reshape(D, HPC * V)
        in_maps.append(
            {
                "xq": xq_pm[b],
                "xk": xk_pm[b],
                "xv": xv_pm[b],
                "wq": _pm_w(wq_c),
                "wk": _pm_w(wk_c),
                "wv": _pm_w(wv_c),
                "wo": wo_pm,
            }
        )
    return in_maps


def run(inputs, trace=False, **spmd_kwargs):
    """Run on 8 cores; returns (full_output [B,S,D] f32, BassKernelResults)."""
    queries = np.asarray(inputs["queries"], np.float32)
    keys = np.asarray(inputs["keys"], np.float32)
    values = np.asarray(inputs["values"], np.float32)
    W_q = np.asarray(inputs["W_q"], np.float32)
    W_k = np.asarray(inputs["W_k"], np.float32)
    W_v = np.asarray(inputs["W_v"], np.float32)
    W_o = np.asarray(inputs["W_o"], np.float32)

    nc = _get_nc()
    in_maps = _prep_in_maps(queries, keys, values, W_q, W_k, W_v, W_o)
    res = run_bass_kernel_spmd(
        nc, in_maps, core_ids=list(range(8)), trace=trace, **spmd_kwargs
    )
    out = np.empty((B, S, D), np.float32)
    half = S * HPC // H  # 1024 output rows per head-group core
    DT = D // 128
    for b in range(B):
        for hg in range(2):
            y_pm = res.results[2 * b + hg]["y"]  # [128, DT*half]
            yt = y_pm.reshape(128, DT, half).transpose(1, 0, 2).reshape(D, half)
            out[b, hg * half : (hg + 1) * half] = yt.T
    return out, res


def kernel(**inputs) -> np.ndarray:
    out, _ = run(inputs, trace=False)
    return out


# revision 27
# speedup vs baseline: 1.0247x; 1.0237x over previous
"""Trainium2 8-core kernel for nn_MultiHeadAttention_83408264889124.

Full inputs in, full output out. Sharding: batch (4) x head-group (2) grid
over 8 NeuronCores — each core computes one batch with 6 of the 12 heads and
produces a partial Y^T = sum_h W_o[h]^T @ O_h^T; the host adds the two
head-group partials per batch (the "all-reduce" of the TP split) and
transposes back. All device work is in transposed layouts so no on-device
transposes are needed:

  Qt = (wq/sqrt(K))^T X^T, Kt = wk^T X^T          [K, S] per head
  St[k,q] = sum_d Kt[d,k] Qt[d,q]                  (2-head row-packed matmuls)
  E = exp(St)  (ScalarE, PSUM->SBUF bf16)
  AV with stationary [V_h | ones*64]: U[0:64] = V^T E, U[64:128] = colsum(E)
  Ot = U[0:64] * recip(U[64:128])                  (VectorE)

The reference does a RAW reshape [B,H,S,V] -> [B,S,H*V] (no transpose) before
W_o, which scrambles (head, seq): with t = S*h + s, output row s' = t//H gets
feature block j = t%H from head h, position s. Because S*HPC/H = 1024 exactly,
each head-group core produces a clean half of the output rows, and the scatter
indices depend only on the LOCAL head index - so one SPMD program works for
all cores. The output projection is Y^T = W_o^T @ G^T with the FULL W_o.

Schedule (the perf-critical part): the ScalarE exp stream is the bottleneck
resource (192 ACTIVATEs x ~1.1us with zero slack). The kernel therefore
minimizes time outside the exp-saturated window:
  - DMA order: wk, xk, wv, xv, wq, xq[c0], xq[c1:], wo; all DRAM tensors are
    host-packed partition-major ([128, ...] with large contiguous runs) so
    DMA runs at full rate.
  - Prologue (overlapped with DMA): PE warm-up burst, K-proj pair0 chunk-by-
    chunk behind the xk chunk DMAs, V-proj all kt behind xv, Q-proj pair0
    chunk0. First exp issues ~27us in.
  - All remaining projections (K/Q pairs 1-2, Q pair0 chunks 1-3) and the
    first output-projection column chunk (W_o c0 depends only on heads 0-2 =
    pairs 0,1) are emitted as single-matmul micro-units interleaved ~2 per kt
    iteration into the attention loops, filling the PE slack under the exp
    stream. Only W_o c1 (+ its Y DMA) remains as tail.
  - Attention-phase filler accumulations use a dedicated 1-bank PSUM pool
    (upool shrunk to 3 bufs to free the bank) so they never collide with the
    psS QK^T/exp rotation.

mask is all-ones for this problem (spec fill="ones") and adds 0 to logits, so
it is not read. Compute dtype bf16 (inputs converted host-side), f32
accumulation; softmax without max-subtraction (logits are O(1) by
construction so exp never overflows).
"""

from contextlib import ExitStack

import numpy as np
import ml_dtypes

import concourse.bacc as bacc
import concourse.bass as bass
import concourse.mybir as mybir
import concourse.tile as tile
from concourse.bass_utils import run_bass_kernel_spmd

BF16 = mybir.dt.bfloat16
F32 = mybir.dt.float32
I32 = mybir.dt.int32
EXP = mybir.ActivationFunctionType.Exp
RECIP_MAGIC = 0x7EF311C3

B, S, D, H, K, V = 4, 2048, 768, 12, 64, 64
HPC = 6  # heads per core
CH = 512  # q chunk


def build_nc(S=S, D=D, HPC=HPC, K=K, CH=CH):
    """Build the per-core Bass program (SPMD: same program on all 8 cores)."""
    assert D % 128 == 0 and S % 128 == 0 and S % CH == 0 and K == 64
    DT = D // 128  # contraction tiles for projections
    KT = S // 128  # key-position tiles
    QC = S // CH  # q chunks
    NP = HPC // 2  # head pairs
    FW = HPC * K  # per-core projection feature width (384)
    assert FW // 128 == NP

    nc = bacc.Bacc("TRN2", target_bir_lowering=False, debug=False, num_devices=8)

    # all DRAM tensors are partition-major: [128, ...] with the row-block
    # index folded into the column axis (host packs them; see _prep_in_maps)
    xq = nc.declare_dram_parameter("xq", [128, DT * S], BF16, isOutput=False)
    xk = nc.declare_dram_parameter("xk", [128, DT * S], BF16, isOutput=False)
    xv = nc.declare_dram_parameter("xv", [128, DT * S], BF16, isOutput=False)
    wq = nc.declare_dram_parameter("wq", [128, DT * FW], BF16, isOutput=False)
    wk = nc.declare_dram_parameter("wk", [128, DT * FW], BF16, isOutput=False)
    wv = nc.declare_dram_parameter("wv", [128, DT * FW], BF16, isOutput=False)
    wo = nc.declare_dram_parameter("wo", [128, DT * D], BF16, isOutput=False)
    SOUT = S * HPC // H  # output rows produced by this core (1024)
    y = nc.declare_dram_parameter("y", [128, DT * SOUT], F32, isOutput=True)

    with tile.TileContext(nc) as tc, ExitStack() as ctx:
        xpool = ctx.enter_context(tc.tile_pool(name="xin", bufs=1))
        wpool = ctx.enter_context(tc.tile_pool(name="w", bufs=1))
        qkpool = ctx.enter_context(tc.tile_pool(name="qk", bufs=1))
        vpool = ctx.enter_context(tc.tile_pool(name="vhat", bufs=1))
        opool = ctx.enter_context(tc.tile_pool(name="ot", bufs=1))
        epool = ctx.enter_context(tc.tile_pool(name="exps", bufs=10))
        rpool = ctx.enter_context(tc.tile_pool(name="rec", bufs=6))
        ucpool = ctx.enter_context(tc.tile_pool(name="uc", bufs=4))
        unpool = ctx.enter_context(tc.tile_pool(name="un", bufs=4))
        ypool = ctx.enter_context(tc.tile_pool(name="yev", bufs=4))
        # PSUM budget (8 banks): psS 2x[128,1024] (4) + U 2x[128,512] (2)
        # + 2-bank filler accumulator apool = 8. U can be 2-deep because the
        # whole accumulator is staged to SBUF in ONE fast copy right after
        # kt15 (recip/normalize then runs from the copy, off the WAR path);
        # apool being 2-deep keeps filler groups from serializing on their
        # own evictions (that WAR chain was the main exp-stream staller).
        pspool = ctx.enter_context(tc.tile_pool(name="ps", bufs=2, space="PSUM"))
        upool = ctx.enter_context(tc.tile_pool(name="us", bufs=2, space="PSUM"))
        apool = ctx.enter_context(tc.tile_pool(name="acc", bufs=2, space="PSUM"))

        def load_w(dram):
            """DRAM [128, n] (partition-major) -> SBUF tile [128, n]."""
            t = wpool.tile(
                [128, dram.shape[1]], BF16, tag=dram.name, name=dram.name + "_sb"
            )
            nc.sync.dma_start(t[:], dram[:, :])
            return t

        xq_sb = xpool.tile([128, DT * S], BF16, tag="xq")
        xk_sb = xpool.tile([128, DT * S], BF16, tag="xk")
        xv_sb = xpool.tile([128, DT * S], BF16, tag="xv")

        def load_x_chunk(t, dram, qc):
            # dram x layout: [128, (qc, n, m)] -- chunk qc is one contiguous
            # [128, DT*CH] block (6 KiB runs per partition)
            t3 = t[:].rearrange("p (n m) -> p n m", m=S)
            d4 = dram[:, :].rearrange("p (q n m) -> p q n m", q=QC, m=CH)
            nc.sync.dma_start(
                t3[:, :, qc * CH : qc * CH + CH],
                d4[:, qc],
            )

        # DMA issue order: xv first (V-proj is the bulk of prologue PE work
        # and runs shadowed by the rest of the load), then the MINIMAL
        # first-exp critical path wk+xk[c0]+wq+xq[c0] (attention kt 0-3 only
        # needs K-proj chunk 0), then the remaining xk/xq chunks in kt/qc
        # consumption order, wo last
        wv_sb = load_w(wv)
        for qc in range(QC):
            load_x_chunk(xv_sb, xv, qc)
        wk_sb = load_w(wk)
        load_x_chunk(xk_sb, xk, 0)
        wq_sb = load_w(wq)
        load_x_chunk(xq_sb, xq, 0)
        for qc in range(1, QC):
            load_x_chunk(xk_sb, xk, qc)
        for qc in range(1, QC):
            load_x_chunk(xq_sb, xq, qc)
        wo_sb = load_w(wo)

        # PE warm-up burst: dependency-free matmuls that run during the
        # initial DMA wait so the HAM clock gate is at 8/8 (2.4 GHz) when
        # the first projection matmuls issue
        wu = wpool.tile([128, 128], BF16, tag="warm", name="warm")
        nc.vector.memset(wu[:], 0.0)
        pswu = pspool.tile([128, 2 * CH], F32, tag="s", name="pswu")
        for _ in range(56):
            nc.tensor.matmul(pswu[:, 0:128], wu[:], wu[:], start=True, stop=True)
        # dummy exp during the DMA wait: pulls the ~2.7us ACT table load
        # (PSEUDO_LOAD_ACT_FUNC_SET rides on the first ACTIVATE) off the
        # first real exp's critical path
        edum = wpool.tile([128, 16], BF16, tag="edum", name="edum")
        nc.scalar.activation(edum[:], pswu[:, 0:16], EXP)

        qt_sb = [
            qkpool.tile([128, S], BF16, tag=f"qt{p}", name=f"qt{p}")
            for p in range(NP)
        ]
        kt_sb = [
            qkpool.tile([128, S], BF16, tag=f"kt{p}", name=f"kt{p}")
            for p in range(NP)
        ]
        # G^T tiles: row 64j+v, col c — g-th tile holds j in {2g, 2g+1}
        gt_sb = [
            opool.tile([128, SOUT], BF16, tag=f"gt{g}", name=f"gt{g}")
            for g in range(D // 128)
        ]
        # vhat[kt]: [128, HPC*128]; head h occupies cols [128h,128h+128) as
        # [V_h (64) | ones (64)] — the ones columns make the AV matmul also
        # produce sum(exp) replicated across partitions 64..127.
        vhat = [
            vpool.tile([128, HPC * 128], BF16, tag=f"vh{k}", name=f"vh{k}")
            for k in range(KT)
        ]

        # Projection / output-projection work is emitted as micro-units
        # (one matmul or one eviction per unit) so it can be interleaved
        # into the attention kt-loop: the PE queue is in-order, so coarse
        # blocks would head-of-line-block the QK^T matmuls that feed the
        # (bottleneck) ScalarE exp stream.
        def acc_group_units(mm_emit, fin_emit, pool_tag, pslice=None):
            st = {}
            pool = pspool if pool_tag == "s" else apool

            def mk(dt):
                def f():
                    if dt == 0:
                        width = 2 * CH if pool_tag == "s" else CH
                        ps = pool.tile([128, width], F32, tag=pool_tag, name="acc")
                        st["ps"] = ps[:, : pslice or CH]
                    mm_emit(st["ps"], dt)

                return f

            return [mk(dt) for dt in range(DT)] + [lambda: fin_emit(st["ps"])]

        def proj_chunk_units(w_sb, x_sb, dst, hp, qc, pool_tag):
            def mm(ps, dt):
                nc.tensor.matmul(
                    ps[:],
                    w_sb[:, dt * FW + hp * 128 : dt * FW + hp * 128 + 128],
                    x_sb[:, dt * S + qc * CH : dt * S + qc * CH + CH],
                    start=(dt == 0),
                    stop=(dt == DT - 1),
                )

            def fin(ps):
                nc.vector.tensor_copy(dst[:, qc * CH : qc * CH + CH], ps[:])

            return acc_group_units(mm, fin, pool_tag)

        def v_proj_units(kt, pool_tag="s"):
            # V projection for ALL heads at once (moving N=FW=384)
            def mm(pv, dt):
                nc.tensor.matmul(
                    pv[:],
                    xv_sb[:, dt * S + kt * 128 : dt * S + kt * 128 + 128],
                    wv_sb[:, dt * FW : dt * FW + FW],
                    start=(dt == 0),
                    stop=(dt == DT - 1),
                )

            def fin(pv):
                dst3 = vhat[kt][:].rearrange("p (h m) -> p h m", m=128)
                nc.vector.tensor_copy(
                    dst3[:, :, 0:64],
                    pv[:].rearrange("p (h m) -> p h m", m=64),
                )
                nc.vector.memset(dst3[:, :, 64:128], 1.0)

            return acc_group_units(mm, fin, pool_tag, pslice=FW)

        def v_proj_units_a(kt):
            return v_proj_units(kt, "a")

        GT = D // 128

        def _wo_dt_units(c0, cw, dt, pool_tag):
            # output projection for y rows [128*dt, 128*dt+128), cols [c0,
            # c0+cw): accumulate Y^T = sum_g W_o[g]^T @ G^T[g] over the GT
            # feature blocks
            def mm(py, g):
                nc.tensor.matmul(
                    py[:],
                    wo_sb[:, g * D + dt * 128 : g * D + dt * 128 + 128],
                    gt_sb[g][:, c0 : c0 + cw],
                    start=(g == 0),
                    stop=(g == GT - 1),
                )

            def fin(py):
                yt = ypool.tile([128, cw], F32, tag="yev", name="yt")
                nc.vector.tensor_copy(yt[:], py[:])
                nc.sync.dma_start(
                    y[:, dt * SOUT + c0 : dt * SOUT + c0 + cw],
                    yt[:],
                )

            st = {}
            pool = pspool if pool_tag == "s" else apool

            def mkmm(g):
                def f():
                    if g == 0:
                        width = 2 * CH if pool_tag == "s" else CH
                        ps = pool.tile([128, width], F32, tag=pool_tag, name="acc")
                        st["ps"] = ps[:, :cw]
                    mm(st["ps"], g)

                return f

            return [mkmm(g) for g in range(GT)] + [lambda: fin(st["ps"])]

        def wo_units(c0, cw, pool_tag):
            units = []
            for dt in range(DT):
                units += _wo_dt_units(c0, cw, dt, pool_tag)
            return units

        # ---- prologue: shadowed by the input DMAs ----
        # V proj kts 0-9 behind the xv loads (the rest go in as fillers:
        # V-proj's ~20us of PE issue time overshoots the xv DMA window)
        VPRE = 10
        for kt in range(VPRE):
            for un in v_proj_units(kt):
                un()
        # K proj pair0 chunk 0 + Q proj pair0 chunk 0: the minimal feed for
        # the first 4 attention kt slots
        for un in proj_chunk_units(wk_sb, xk_sb, kt_sb[0], 0, 0, "s"):
            un()
        for un in proj_chunk_units(wq_sb, xq_sb, qt_sb[0], 0, 0, "s"):
            un()

        # ---- filler queue: everything else, with CORRECTNESS deadlines.
        # Tile's hazard tracking is emission-ordered: a read emitted before
        # its producer sees garbage. Every filler unit therefore carries the
        # slot index it must be emitted by (the slot just before its first
        # reader's emission, which includes the one-slot QK^T lookahead);
        # the slot loop force-drains overdue units at slot start and
        # otherwise pops ahead at a steady rate. ----
        fillers = []  # (deadline_slot, unit), kept deadline-sorted

        def addf(deadline, units):
            fillers.extend((deadline, u) for u in units)

        for c in range(1, QC):
            addf(4 * c - 2, proj_chunk_units(wk_sb, xk_sb, kt_sb[0], 0, c, "a"))
        for kt in range(VPRE, KT):
            addf(kt, v_proj_units_a(kt))
        for c in range(1, QC):
            addf(16 * c - 2, proj_chunk_units(wq_sb, xq_sb, qt_sb[0], 0, c, "a"))
        # next pairs' K/Q proj: deadline-spread across the preceding pair's
        # slots so the force-drain never dumps a big burst in one slot
        for hp in (1, 2):
            base = 64 * (hp - 1) + 20
            for i, c in enumerate(range(QC)):
                addf(
                    base + 5 * i,
                    proj_chunk_units(wk_sb, xk_sb, kt_sb[hp], hp, c, "a"),
                )
            for i, c in enumerate(range(QC)):
                addf(
                    base + 5 * (4 + i),
                    proj_chunk_units(wq_sb, xq_sb, qt_sb[hp], hp, c, "a"),
                )
        fillers.sort(key=lambda du: du[0])

        late_fillers = []  # W_o chunks: gated on the gt scatters they read

        def pop_fillers(n):
            for _ in range(n):
                if fillers:
                    fillers.pop(0)[1]()
                elif late_fillers:
                    late_fillers.pop()()

        def emit_qkt(hp, qc, kt):
            psS = pspool.tile([128, 2 * CH], F32, tag="s", name="psS")
            # row-packed pair: head A rows 0-63, head B rows 64-127; the two
            # matmuls land on disjoint PE row-groups and run concurrently
            nc.tensor.matmul(
                psS[:, 0:CH],
                kt_sb[hp][0:64, kt * 128 : kt * 128 + 128],
                qt_sb[hp][0:64, qc * CH : qc * CH + CH],
                start=True,
                stop=True,
            )
            nc.tensor.matmul(
                psS[:, CH : 2 * CH],
                kt_sb[hp][64:128, kt * 128 : kt * 128 + 128],
                qt_sb[hp][64:128, qc * CH : qc * CH + CH],
                start=True,
                stop=True,
            )
            return psS

        scat_q = []  # deferred gt-scatter copies (DVE), drained ~3/slot

        def emit_normalize(u_a, u_b, hp, qc, defer=True):
            # Newton reciprocal of the replicated exp-sums in rows 64..127
            # (magic seed + 2 NR passes; w holds -1/l at ~1e-5 rel err).
            # The two heads' chains are interleaved per-op so the DVE
            # pipeline stays full (the chain is serially dependent per head)
            # FIRST: stage each PSUM accumulator to SBUF in one full-tile
            # copy — this is the only reader of u, so the 2-deep u pool
            # recycles ~0.7us after kt15 instead of after the whole chain
            pair = ((u_a, 2 * hp), (u_b, 2 * hp + 1))
            ucs = []
            for u, hl in pair:
                # partition-shifting reads are only legal from PSUM, so
                # stage the two halves into partition-0-based tiles
                uv = ucpool.tile([64, CH], F32, tag="uc", name="uv")
                nc.vector.tensor_copy(uv[:], u[0:64, :])
                ud = ucpool.tile([64, CH], F32, tag="uc", name="ud")
                nc.vector.tensor_copy(ud[:], u[64:128, :])
                ucs.append((uv, ud))
            rs, ts, ws, uns = [], [], [], []
            for uv, ud in ucs:
                r = rpool.tile([64, CH], F32, tag="rec", name="r")
                nc.vector.tensor_scalar(
                    r[:].bitcast(I32), ud[:].bitcast(I32),
                    RECIP_MAGIC, -1,
                    mybir.AluOpType.subtract, mybir.AluOpType.mult,
                )
                rs.append(r)
            for (uv, ud), r in zip(ucs, rs):
                t = rpool.tile([64, CH], F32, tag="rec", name="t")
                nc.vector.tensor_mul(t[:], ud[:], r[:])
                ts.append(t)
            for r, t in zip(rs, ts):
                w = rpool.tile([64, CH], F32, tag="rec", name="w")
                nc.vector.scalar_tensor_tensor(
                    w[:], t[:], 2.0, r[:],
                    mybir.AluOpType.subtract, mybir.AluOpType.mult,
                )
                ws.append(w)
            for (uv, ud), w in zip(ucs, ws):
                un = unpool.tile([64, CH], F32, tag="un", name="un")
                nc.vector.scalar_tensor_tensor(
                    un[:], uv[:], -1.0, w[:],
                    mybir.AluOpType.mult, mybir.AluOpType.mult,
                )
                uns.append(un)

            # scatter: Ot[v, s] -> G^T[64j+v, c] with j=(S*hl+s)%H,
            # c=(S*hl+s)//H; strided in s (step H). Deferred into the next
            # qc's slots so the 24-copy DVE burst never backs up the filler
            # evictions (whose PSUM WAR would stall the PE queue)
            for (u, hl), un in zip(pair, uns):
                cq0 = qc * CH
                for j in range(H):
                    s0 = (j - S * hl) % H
                    m0 = max(0, -(-(cq0 - s0) // H))
                    s_st = s0 + H * m0
                    if s_st >= cq0 + CH:
                        continue
                    count = (cq0 + CH - 1 - s_st) // H + 1
                    o = s_st - cq0
                    c_st = (S * hl + s_st) // H
                    sl = slice(o, o + H * (count - 1) + 1, H)

                    def cp(un=un, j=j, c_st=c_st, count=count, sl=sl):
                        nc.vector.tensor_copy(
                            gt_sb[j // 2][
                                64 * (j % 2) : 64 * (j % 2) + 64,
                                c_st : c_st + count,
                            ],
                            un[:, sl],
                        )

                    if defer:
                        scat_q.append(cp)
                    else:
                        cp()

        # ---- attention: flat kt stream with one-slot QK^T lookahead so the
        # next qc's logits are already in flight when a qc ends (keeps the
        # exp stream gap-free across qc boundaries) ----
        slots = [
            (hp, qc, kt)
            for hp in range(NP)
            for qc in range(QC)
            for kt in range(KT)
        ]
        # W_o 256-col chunks become available as the gt bands they read
        # finish scattering: chunk c is gated on (pair, qc) per the scramble
        # geometry; (slot_index -> chunk) emission gates (one qc of margin
        # for the deferred scatters):
        WCW = 256
        wo_gate = {
            1 * QC * KT + 1 * KT: 0,  # [0:256)    during pair1 (needs pair0)
            2 * QC * KT + 1 * KT: 1,  # [256:512)  during pair2 (needs p1)
            2 * QC * KT + 3 * KT: 2,  # [512:768)  during p2 qc3 (hl4 qc0-1)
        }
        psS_cur = emit_qkt(*slots[0])
        u_a = u_b = None
        for g, (hp, qc, kt) in enumerate(slots):
            # correctness: emit every filler whose first reader sits in this
            # slot (or the next slot's QK^T) before any attention unit
            while fillers and fillers[0][0] <= g:
                fillers.pop(0)[1]()
            if g in wo_gate:
                # prepend reversed (consumed via pop() from the end)
                late_fillers[:0] = reversed(wo_units(wo_gate[g] * WCW, WCW, "a"))
            if kt == 0:
                u_a = upool.tile([128, CH], F32, tag="u", name="ua")
                u_b = upool.tile([128, CH], F32, tag="u", name="ub")
            es = epool.tile([128, 2 * CH], BF16, tag="es", name="es")
            nc.scalar.activation(es[:], psS_cur[:], EXP)
            if g + 1 < len(slots):
                psS_cur = emit_qkt(*slots[g + 1])
            nc.tensor.matmul(
                u_a[:],
                vhat[kt][:, 256 * hp : 256 * hp + 128],
                es[:, 0:CH],
                start=(kt == 0),
                stop=(kt == KT - 1),
            )
            nc.tensor.matmul(
                u_b[:],
                vhat[kt][:, 256 * hp + 128 : 256 * hp + 256],
                es[:, CH : 2 * CH],
                start=(kt == 0),
                stop=(kt == KT - 1),
            )
            if kt == KT - 1:
                emit_normalize(u_a, u_b, hp, qc, defer=(g + 1 < len(slots)))
            # drain deferred scatter copies (DVE; no PE cost)
            for _ in range(3):
                if scat_q:
                    scat_q.pop(0)()
            # PE fillers: stay ahead in the early (V-proj/K-proj) era, and
            # throttle at qc boundaries so PSUM-WAR chains never sit in
            # front of the next qc's QK^T
            if kt in (0, KT - 1):
                pop_fillers(1)
            elif fillers:
                pop_fillers(4 if g < 16 else 2)
            else:
                pop_fillers(3)  # wo late-filler era: M=256 units are cheap

        # tail: leftover scatters + fillers, then W_o [768:1024) on the
        # roomy "s" pool (psS is free now -> no 1-buf WAR chain)
        while scat_q:
            scat_q.pop(0)()
        while fillers or late_fillers:
            pop_fillers(1)
        for un in wo_units(3 * WCW, WCW, "s"):
            un()

    nc.compile()
    return nc


_NC_CACHE = None


def _get_nc():
    global _NC_CACHE
    if _NC_CACHE is None:
        _NC_CACHE = build_nc()
    return _NC_CACHE


def _pm_x(a):
    # [S, D] f32 -> X^T partition-major [128, QC, DT, CH] -> [128, DT*S]
    bf = ml_dtypes.bfloat16
    QC = S // CH
    DT = D // 128
    t = a.T.astype(bf)  # [D, S]
    return (
        t.reshape(DT, 128, QC, CH).transpose(1, 2, 0, 3).reshape(128, DT * S)
    )


def _pm_w(w):
    # [D, F] -> partition-major [128, DT*F]
    bf = ml_dtypes.bfloat16
    DT = D // 128
    F = w.shape[1]
    return w.astype(bf).reshape(DT, 128, F).transpose(1, 0, 2).reshape(128, DT * F)


def _prep_in_maps(queries, keys, values, W_q, W_k, W_v, W_o):
    scale = np.float32(1.0 / np.sqrt(K))
    in_maps = []
    xq_pm = [_pm_x(queries[b]) for b in range(B)]
    xk_pm = [_pm_x(keys[b]) for b in range(B)]
    xv_pm = [_pm_x(values[b]) for b in range(B)]
    wo_pm = _pm_w(W_o)  # full W_o: the raw-reshape scramble touches all rows
    for core in range(8):
        b, hg = divmod(core, 2)
        h0 = hg * HPC
        wq_c = (W_q[h0 : h0 + HPC] * scale).transpose(1, 0, 2).reshape(D, HPC * K)
        wk_c = W_k[h0 : h0 + HPC].transpose(1, 0, 2).reshape(D, HPC * K)
        wv_c = W_v[h0 : h0 + HPC].transpose(1, 0, 2).reshape(D, HPC * V)
        in_maps.append(
            {
                "xq": xq_pm[b],
                "xk": xk_pm[b],
                "xv": xv_pm[b],
                "wq": _pm_w(wq_c),
                "wk": _pm_w(wk_c),
                "wv": _pm_w(wv_c),
                "wo": wo_pm,
            }
        )
    return in_maps


def run(inputs, trace=False, **spmd_kwargs):
    """Run on 8 cores; returns (full_output [B,S,D] f32, BassKernelResults)."""
    queries = np.asarray(inputs["queries"], np.float32)
    keys = np.asarray(inputs["keys"], np.float32)
    values = np.asarray(inputs["values"], np.float32)
    W_q = np.asarray(inputs["W_q"], np.float32)
    W_k = np.asarray(inputs["W_k"], np.float32)
    W_v = np.asarray(inputs["W_v"], np.float32)
    W_o = np.asarray(inputs["W_o"], np.float32)

    nc = _get_nc()
    in_maps = _prep_in_maps(queries, keys, values, W_q, W_k, W_v, W_o)
    res = run_bass_kernel_spmd(
        nc, in_maps, core_ids=list(range(8)), trace=trace, **spmd_kwargs
    )
    out = np.empty((B, S, D), np.float32)
    half = S * HPC // H  # 1024 output rows per head-group core
    DT = D // 128
    for b in range(B):
        for hg in range(2):
            y_pm = res.results[2 * b + hg]["y"]  # [128, DT*half]
            yt = y_pm.reshape(128, DT, half).transpose(1, 0, 2).reshape(D, half)
            out[b, hg * half : (hg + 1) * half] = yt.T
    return out, res


def kernel(**inputs) -> np.ndarray:
    out, _ = run(inputs, trace=False)
    return out
